# revision 7
# baseline (speedup 1.0000x reference)
"""Trainium2 Bass kernel for nn_MinMaxQuantizer (per-channel symmetric log_2 quantizer).

Math (per row c of x[C, D], half = 2**(n_bits-1)):
    rmax    = max(|x[c, :]|)
    max_val = floor(log2(rmax) + 0.5)                 # round-half-up of log2
    z       = max_val - (half - 1)                    # min kept exponent
    e       = round(log2(|x|))                        # per element
    out     = sign(x) * 2^e   if e >= z else 0

v4: carry-into-exponent trick.  Rounding log2 to the nearest integer ==
"round the exponent up iff mantissa_bits >= 0x3504F4" (the sqrt(2) boundary;
irrational, so ties cannot occur).  Adding 0x800000 - 0x3504F4 to the raw fp32
bits carries into the exponent field exactly when the mantissa is above the
boundary:

    y   = bits(x) + 0x4AFB0C          # exponent field of y is e, sign kept
    p   = y & 0xFF800000              # bits of sign(x) * 2^e
    q   = int16(p.f32 * 2^-z)         # truncation zeroes |v| < 1 (e < z)
    out = f32(q) * 2^z

The add runs on the Activation engine (Copy with a float bias); its fp32
internal pipeline rounds the 32-bit sum to 24-bit mantissa, which can flip
the round-up decision for the ~1.5e-5 of elements within 64 ULP of the
mantissa boundary (measured rel err 2e-3, gate is 2e-2).  The row max is
reduced over y directly: the exponent field of max|y| is exactly max(e)
because |y| < 2^(e+1).  Per-row params are derived from those bits with tiny
u32 ops (bits(2^z) + bits(2^-z) = 254<<23).

Engine split per [128, W] chunk — only empirically-fast primitives (gpsimd
bulk tensor ops run at DSP speed, and DVE TENSOR_SCALAR with 16-bit in0 hits
a ~14x slow path, so both are avoided):
    ACT:  y = x + carry (u32 Copy+bias), out = f32(q)*2^z (Copy+scale AP)
    DVE:  abs-max reduce over y.f32, u32 mask, q = p.f32 * 2^-z (AP scalar)
    Pool(gpsimd): output DMA triggers only (SWDGE), so they never queue
          behind input DMA triggers on Sync
    Sync: input DMA triggers

Sharding: rows 4096 -> 8 cores x 512 rows, zero communication.
"""

import sys

import numpy as np

_REPO = "/opt/trn_rl_repo"

N_ROWS = 4096
N_COLS = 11008
N_CORES = 8
ROWS_PER_CORE = N_ROWS // N_CORES  # 512
P = 128
N_SLAB = ROWS_PER_CORE // P  # 4
N_CH = 4
W = N_COLS // N_CH  # 2752

_CARRY = 0x00800000 - 0x3504F4  # 0x4AFB0C: carry bumps exponent iff m >= 0x3504F4
_EXP_MASK = 0x7F800000
_SIGNEXP_MASK = 0xFF800000
_INV_CONST = float(254 << 23)  # bits(2^z) + bits(2^-z)


def _ensure_path():
    if _REPO not in sys.path:
        sys.path.insert(0, _REPO)


def _build(n_bits: int):
    _ensure_path()
    import concourse.bacc as bacc
    import concourse.mybir as mybir
    import concourse.tile as tile

    dt = mybir.dt
    Alu = mybir.AluOpType
    Act = mybir.ActivationFunctionType
    X = mybir.AxisListType.X

    half_sub = float((2 ** (n_bits - 1) - 1) << 23)  # bits offset: max_val -> z

    nc = bacc.Bacc("TRN2", target_bir_lowering=False, debug=False, num_devices=N_CORES)
    x_ext = nc.dram_tensor("x", [ROWS_PER_CORE, N_COLS], dt.float32, kind="ExternalInput")
    out_ext = nc.dram_tensor("out", [ROWS_PER_CORE, N_COLS], dt.float32, kind="ExternalOutput")

    with tile.TileContext(nc) as tc:
        with (
            tc.tile_pool(name="xp", bufs=3) as xp,
            tc.tile_pool(name="yp", bufs=7) as yp,
            tc.tile_pool(name="pp", bufs=2) as pp,
            tc.tile_pool(name="qp", bufs=3) as qp,
            tc.tile_pool(name="op", bufs=4) as op,
            tc.tile_pool(name="st", bufs=2) as st,
        ):
            def load_chunks(s):
                """DMA in, +carry (ACT), abs-max partials (DVE)."""
                r0 = s * P
                yts = []
                rpart = st.tile([P, N_CH], dt.float32, tag="rpart", name=f"rpart{s}")
                for j in range(N_CH):
                    c0 = j * W
                    xt = xp.tile([P, W], dt.uint32, tag="x", name=f"x{s}_{j}")
                    nc.sync.dma_start(
                        out=xt[:], in_=x_ext[r0 : r0 + P, c0 : c0 + W].bitcast(dt.uint32)
                    )
                    yt = yp.tile([P, W], dt.uint32, tag="y", name=f"y{s}_{j}")
                    nc.scalar.activation(
                        out=yt[:], in_=xt[:], func=Act.Copy, bias=float(_CARRY), scale=1.0,
                    )
                    nc.vector.tensor_reduce(
                        out=rpart[:, j : j + 1], in_=yt[:].bitcast(dt.float32), axis=X,
                        op=Alu.max, apply_absolute_value=True,
                    )
                    yts.append(yt)
                return yts, rpart

            def row_params(s, rpart):
                """bits(2^max_val) -> per-row scale APs 2^-z and 2^z (f32 views)."""
                rmax = st.tile([P, 1], dt.float32, tag="rmax", name=f"rmax{s}")
                nc.vector.tensor_reduce(out=rmax[:], in_=rpart[:], axis=X, op=Alu.max)
                eb = st.tile([P, 1], dt.uint32, tag="eb", name=f"eb{s}")
                nc.vector.tensor_scalar(
                    out=eb[:], in0=rmax[:].bitcast(dt.uint32),
                    scalar1=_EXP_MASK, scalar2=None, op0=Alu.bitwise_and,
                )
                zbits = st.tile([P, 1], dt.uint32, tag="zbits", name=f"zbits{s}")
                nc.vector.tensor_scalar(
                    out=zbits[:], in0=eb[:], scalar1=half_sub, scalar2=None,
                    op0=Alu.subtract,
                )
                ihb = st.tile([P, 1], dt.uint32, tag="ihb", name=f"ihb{s}")
                nc.vector.tensor_scalar(
                    out=ihb[:], in0=zbits[:], scalar1=-1.0, scalar2=_INV_CONST,
                    op0=Alu.mult, op1=Alu.add,
                )
                return ihb[:].bitcast(dt.float32), zbits[:].bitcast(dt.float32)

            def quant_chunks(s, yts, ihzf, zf):
                r0 = s * P
                for j in range(N_CH):
                    c0 = j * W
                    pt = pp.tile([P, W], dt.uint32, tag="p", name=f"p{s}_{j}")
                    nc.vector.tensor_scalar(
                        out=pt[:], in0=yts[j][:], scalar1=_SIGNEXP_MASK, scalar2=None,
                        op0=Alu.bitwise_and,
                    )
                    qt = qp.tile([P, W], dt.int16, tag="q", name=f"q{s}_{j}")
                    nc.vector.tensor_scalar(
                        out=qt[:], in0=pt[:].bitcast(dt.float32), scalar1=ihzf,
                        scalar2=None, op0=Alu.mult,
                    )
                    ot = op.tile([P, W], dt.float32, tag="o", name=f"o{s}_{j}")
                    nc.scalar.activation(
                        out=ot[:], in_=qt[:], func=Act.Copy, bias=0.0, scale=zf,
                    )
                    nc.gpsimd.dma_start(out=out_ext[r0 : r0 + P, c0 : c0 + W], in_=ot[:])

            cur, cur_rpart = load_chunks(0)
            for s in range(N_SLAB):
                ihzf, zf = row_params(s, cur_rpart)
                quant_chunks(s, cur, ihzf, zf)
                if s + 1 < N_SLAB:
                    cur, cur_rpart = load_chunks(s + 1)

    nc.compile()
    return nc


def kernel(x, n_bits):
    _ensure_path()
    from concourse.bass_utils import run_bass_kernel_spmd

    x = np.ascontiguousarray(np.asarray(x, dtype=np.float32))
    assert x.shape == (N_ROWS, N_COLS), x.shape
    nb = int(np.asarray(n_bits))

    nc = _build(nb)
    in_maps = [
        {"x": x[i * ROWS_PER_CORE : (i + 1) * ROWS_PER_CORE]} for i in range(N_CORES)
    ]
    res = run_bass_kernel_spmd(nc, in_maps, list(range(N_CORES)))
    return np.concatenate([res.results[i]["out"] for i in range(N_CORES)], axis=0)


# revision 8
# speedup vs baseline: 1.0988x; 1.0988x over previous
"""Trainium2 Bass kernel for nn_MinMaxQuantizer (per-channel symmetric log_2 quantizer).

Math (per row c of x[C, D], half = 2**(n_bits-1)):
    rmax    = max(|x[c, :]|)
    max_val = floor(log2(rmax) + 0.5)                 # round-half-up of log2
    z       = max_val - (half - 1)                    # min kept exponent
    e       = round(log2(|x|))                        # per element
    out     = sign(x) * 2^e   if e >= z else 0

v4: carry-into-exponent trick.  Rounding log2 to the nearest integer ==
"round the exponent up iff mantissa_bits >= 0x3504F4" (the sqrt(2) boundary;
irrational, so ties cannot occur).  Adding 0x800000 - 0x3504F4 to the raw fp32
bits carries into the exponent field exactly when the mantissa is above the
boundary:

    y   = bits(x) + 0x4AFB0C          # exponent field of y is e, sign kept
    p   = y & 0xFF800000              # bits of sign(x) * 2^e
    q   = int16(p.f32 * 2^-z)         # truncation zeroes |v| < 1 (e < z)
    out = f32(q) * 2^z

The add runs on the Activation engine (Copy with a float bias); its fp32
internal pipeline rounds the 32-bit sum to 24-bit mantissa, which can flip
the round-up decision for the ~1.5e-5 of elements within 64 ULP of the
mantissa boundary (measured rel err 2e-3, gate is 2e-2).  The row max is
reduced over y directly: the exponent field of max|y| is exactly max(e)
because |y| < 2^(e+1).  Per-row params are derived from those bits with tiny
u32 ops (bits(2^z) + bits(2^-z) = 254<<23).

Engine split per [128, W] chunk — only empirically-fast primitives (gpsimd
bulk tensor ops run at DSP speed, and DVE TENSOR_SCALAR with 16-bit in0 hits
a ~14x slow path, so both are avoided):
    ACT:  y = x + carry (u32 Copy+bias), out = f32(q)*2^z (Copy+scale AP)
    DVE:  abs-max reduce over y.f32, u32 mask, q = p.f32 * 2^-z (AP scalar)
    Pool(gpsimd): output DMA triggers only (SWDGE), so they never queue
          behind input DMA triggers on Sync
    Sync: input DMA triggers

Sharding: rows 4096 -> 8 cores x 512 rows, zero communication.
"""

import sys

import numpy as np

_REPO = "/opt/trn_rl_repo"

N_ROWS = 4096
N_COLS = 11008
N_CORES = 8
ROWS_PER_CORE = N_ROWS // N_CORES  # 512
P = 128
N_SLAB = ROWS_PER_CORE // P  # 4
N_CH = 4
W = N_COLS // N_CH  # 2752

_CARRY = 0x00800000 - 0x3504F4  # 0x4AFB0C: carry bumps exponent iff m >= 0x3504F4
_EXP_MASK = 0x7F800000
_SIGNEXP_MASK = 0xFF800000
_INV_CONST = float(254 << 23)  # bits(2^z) + bits(2^-z)


def _ensure_path():
    if _REPO not in sys.path:
        sys.path.insert(0, _REPO)


def _build(n_bits: int):
    _ensure_path()
    import concourse.bacc as bacc
    import concourse.mybir as mybir
    import concourse.tile as tile

    dt = mybir.dt
    Alu = mybir.AluOpType
    Act = mybir.ActivationFunctionType
    X = mybir.AxisListType.X

    half_sub = float((2 ** (n_bits - 1) - 1) << 23)  # bits offset: max_val -> z

    nc = bacc.Bacc("TRN2", target_bir_lowering=False, debug=False, num_devices=N_CORES)
    x_ext = nc.dram_tensor("x", [ROWS_PER_CORE, N_COLS], dt.float32, kind="ExternalInput")
    out_ext = nc.dram_tensor("out", [ROWS_PER_CORE, N_COLS], dt.float32, kind="ExternalOutput")

    with tile.TileContext(nc) as tc:
        with (
            tc.tile_pool(name="xp", bufs=6) as xp,
            tc.tile_pool(name="yp", bufs=6) as yp,
            tc.tile_pool(name="pp", bufs=2) as pp,
            tc.tile_pool(name="qp", bufs=2) as qp,
            tc.tile_pool(name="op", bufs=3) as op,
            tc.tile_pool(name="st", bufs=2) as st,
        ):
            def load_chunks(s):
                """DMA in, +carry (ACT), abs-max partials (DVE)."""
                r0 = s * P
                yts = []
                rpart = st.tile([P, N_CH], dt.float32, tag="rpart", name=f"rpart{s}")
                for j in range(N_CH):
                    c0 = j * W
                    xt = xp.tile([P, W], dt.uint32, tag="x", name=f"x{s}_{j}")
                    nc.sync.dma_start(
                        out=xt[:], in_=x_ext[r0 : r0 + P, c0 : c0 + W].bitcast(dt.uint32)
                    )
                    yt = yp.tile([P, W], dt.uint32, tag="y", name=f"y{s}_{j}")
                    nc.scalar.activation(
                        out=yt[:], in_=xt[:], func=Act.Copy, bias=float(_CARRY), scale=1.0,
                    )
                    nc.vector.tensor_reduce(
                        out=rpart[:, j : j + 1], in_=yt[:].bitcast(dt.float32), axis=X,
                        op=Alu.max, apply_absolute_value=True,
                    )
                    yts.append(yt)
                return yts, rpart

            def row_params(s, rpart):
                """bits(2^max_val) -> per-row scale APs 2^-z and 2^z (f32 views)."""
                rmax = st.tile([P, 1], dt.float32, tag="rmax", name=f"rmax{s}")
                nc.vector.tensor_reduce(out=rmax[:], in_=rpart[:], axis=X, op=Alu.max)
                eb = st.tile([P, 1], dt.uint32, tag="eb", name=f"eb{s}")
                nc.vector.tensor_scalar(
                    out=eb[:], in0=rmax[:].bitcast(dt.uint32),
                    scalar1=_EXP_MASK, scalar2=None, op0=Alu.bitwise_and,
                )
                zbits = st.tile([P, 1], dt.uint32, tag="zbits", name=f"zbits{s}")
                nc.vector.tensor_scalar(
                    out=zbits[:], in0=eb[:], scalar1=half_sub, scalar2=None,
                    op0=Alu.subtract,
                )
                ihb = st.tile([P, 1], dt.uint32, tag="ihb", name=f"ihb{s}")
                nc.vector.tensor_scalar(
                    out=ihb[:], in0=zbits[:], scalar1=-1.0, scalar2=_INV_CONST,
                    op0=Alu.mult, op1=Alu.add,
                )
                return ihb[:].bitcast(dt.float32), zbits[:].bitcast(dt.float32)

            def quant_chunks(s, yts, ihzf, zf):
                r0 = s * P
                for j in range(N_CH):
                    c0 = j * W
                    pt = pp.tile([P, W], dt.uint32, tag="p", name=f"p{s}_{j}")
                    nc.vector.tensor_scalar(
                        out=pt[:], in0=yts[j][:], scalar1=_SIGNEXP_MASK, scalar2=None,
                        op0=Alu.bitwise_and,
                    )
                    qt = qp.tile([P, W], dt.int16, tag="q", name=f"q{s}_{j}")
                    nc.vector.tensor_scalar(
                        out=qt[:], in0=pt[:].bitcast(dt.float32), scalar1=ihzf,
                        scalar2=None, op0=Alu.mult,
                    )
                    ot = op.tile([P, W], dt.float32, tag="o", name=f"o{s}_{j}")
                    nc.scalar.activation(
                        out=ot[:], in_=qt[:], func=Act.Copy, bias=0.0, scale=zf,
                    )
                    nc.gpsimd.dma_start(out=out_ext[r0 : r0 + P, c0 : c0 + W], in_=ot[:])

            cur, cur_rpart = load_chunks(0)
            for s in range(N_SLAB):
                ihzf, zf = row_params(s, cur_rpart)
                quant_chunks(s, cur, ihzf, zf)
                if s + 1 < N_SLAB:
                    cur, cur_rpart = load_chunks(s + 1)

    nc.compile()
    return nc


def kernel(x, n_bits):
    _ensure_path()
    from concourse.bass_utils import run_bass_kernel_spmd

    x = np.ascontiguousarray(np.asarray(x, dtype=np.float32))
    assert x.shape == (N_ROWS, N_COLS), x.shape
    nb = int(np.asarray(n_bits))

    nc = _build(nb)
    in_maps = [
        {"x": x[i * ROWS_PER_CORE : (i + 1) * ROWS_PER_CORE]} for i in range(N_CORES)
    ]
    res = run_bass_kernel_spmd(nc, in_maps, list(range(N_CORES)))
    return np.concatenate([res.results[i]["out"] for i in range(N_CORES)], axis=0)
